# revision 65
# baseline (speedup 1.0000x reference)
"""Distributed GQA attention block (dense transformer) on 8 TRN2 NeuronCores.

Strategy: tensor-parallel over heads. Each core owns 4 query heads + 1 KV head
(GQA group). x^T is replicated; Q/K/V projections, RoPE, scores, softmax and
the attention output all stay in "transposed" layout (feature dim on SBUF
partitions, sequence on the free dim) so no on-device transposes are needed.
The per-core attention outputs are exchanged with AllToAll collectives (each
core keeps a 256-row slice of the sequence), then each core computes its slice
of the output projection against the full (replicated, pre-tiled) wo. The host
concatenates the 8 row slices.

All matmuls run in bf16 with fp32 PSUM accumulation; softmax exp runs in fp32
on the scalar engine (no max-subtraction needed: |scores*scale| <~ 12).

Schedule notes:
- heads are software-pipelined: the sums/AV matmuls of head h-1 are emitted
  after the score matmuls of head h, so the scalar engine's exp of head h
  overlaps PE work of head h-1.
- the AllToAll is split: heads 0-2 exchange while head 3 computes; head 3's
  exchange is covered by the first 3/4 of the output-projection matmuls
  (the wo contraction order is host-permuted to put head-3 blocks last).
"""

import numpy as np
import ml_dtypes

import concourse.bass as bass
import concourse.mybir as mybir
import concourse.tile as tile
from concourse import bacc
from concourse import bass_utils

F32 = mybir.dt.float32
BF16 = mybir.dt.bfloat16

# Problem shape (hardcoded per harness contract).
L = 2048          # sequence length
D = 4096          # model dim
DH = 128          # head dim
NHEADS = 32
NKV = 8
NCORES = 8
HQ = NHEADS // NCORES      # 4 query heads per core
ROPE_THETA = 10000.0
SCALE = DH ** -0.5

ND = D // 128              # 32 contraction chunks over model dim
NLC = L // 512             # 4 free-dim chunks of 512 over sequence
NJ = L // 128              # 16 key chunks of 128
NI = L // 512              # 4 query chunks of 512
IS = L // NCORES           # 256: per-core output row slice

# AllToAll groups: heads {0,1} fly during head-2 compute, {2} during head-3,
# {3} is covered by the first 3/4 of the output projection.
A2A_GROUPS = [(0, 1), (2,), (3,)]

_cached = {}


def build_kernel(debug=False):
    nc = bacc.Bacc(num_devices=NCORES)

    xT = nc.dram_tensor("xT", [D, L], BF16, kind="ExternalInput")
    # 6 head-slots in compute order (k, q0, v, q1, q2, q3), each pre-tiled to
    # [128 partitions, 32*128]: [:, dc*128:(dc+1)*128] is dim-chunk dc.
    wqkv = nc.dram_tensor("wqkv", [6 * 128, ND * 128], BF16, kind="ExternalInput")
    # wo pre-tiled: row (do*8+tq) is a fully contiguous [128, 4*512] SBUF
    # image covering hd-chunks perm[4tq..4tq+3] for out-column chunk do
    # (perm = A2A arrival order).
    wo = nc.dram_tensor("wo", [8 * 8, 128 * 4 * 512], BF16, kind="ExternalInput")
    cosT = nc.dram_tensor("cosT", [128, L], F32, kind="ExternalInput")
    sinT = nc.dram_tensor("sinT", [128, L], F32, kind="ExternalInput")  # sign-folded
    out = nc.dram_tensor("out", [IS, D], F32, kind="ExternalOutput")
    if debug:
        dbg_qk = nc.dram_tensor("dbg_qk", [5 * 128, L], BF16, kind="ExternalOutput")
        dbg_v = nc.dram_tensor("dbg_v", [NJ * 128, DH], BF16, kind="ExternalOutput")
        dbg_og = nc.dram_tensor("dbg_og", [NCORES * HQ * DH, IS], BF16, kind="ExternalOutput")

    swap_mask = []
    for i in range(16):
        swap_mask += [2 * i + 1, 2 * i]

    # slot order in wqkv / processing: k, q0, v, q1, q2, q3
    SLOT_K, SLOT_Q0, SLOT_V = 0, 1, 2
    slot_of_head = [1, 3, 4, 5]  # q0..q3

    with tile.TileContext(nc) as tc:
        with (
            tc.tile_pool(name="const", bufs=1) as cpool,
            tc.tile_pool(name="persist", bufs=1) as ppool,
            tc.tile_pool(name="dram", bufs=1, space="DRAM") as dram,
        ):
            ones_bc = cpool.tile([128, 128], BF16)
            nc.vector.memset(ones_bc[:], 1.0)

            # Roped K^T + Q^T (4 heads), bf16, [head_dim=128, L]
            qk_rope = [ppool.tile([128, L], BF16, name=f"qkrope{s}") for s in range(5)]
            krope = qk_rope[0]
            qrope = [qk_rope[1], qk_rope[2], qk_rope[3], qk_rope[4]]
            rope_dst = {SLOT_K: krope, 3: qrope[1], 4: qrope[2], 5: qrope[3],
                        SLOT_Q0: qrope[0]}
            # V in [seq, head_dim] layout: 16 chunks of [128, 128]
            v_sb = [ppool.tile([128, DH], BF16, name=f"vsb{j}") for j in range(NJ)]

            # ---------------- Phase 1: projections + rope ----------------
            with (
                tc.tile_pool(name="tbl", bufs=1) as tblpool,
                tc.tile_pool(name="wq", bufs=1) as wpool,
                tc.tile_pool(name="xt", bufs=18) as xtpool,
                tc.tile_pool(name="p1psum", bufs=1, space="PSUM") as p1ps,
                tc.tile_pool(name="ropework", bufs=3) as rwork,
            ):
                # weight/table loads go through gpsimd's queue so the xt loads
                # on the sync queue aren't stuck behind them at startup
                cos_sb = tblpool.tile([128, L], F32)
                sin_sb = tblpool.tile([128, L], F32)
                nc.gpsimd.dma_start(cos_sb[:], cosT[:])
                nc.gpsimd.dma_start(sin_sb[:], sinT[:])
                w_sb = []
                for s in range(6):
                    wt = wpool.tile([128, ND * 128], BF16, name=f"w{s}")
                    if s < 2:
                        # chunked so the first matmuls unblock early
                        for q in range(4):
                            nc.gpsimd.dma_start(
                                wt[:, bass.ts(q, ND * 32)],
                                wqkv[s * 128:(s + 1) * 128, bass.ts(q, ND * 32)],
                            )
                    else:
                        nc.gpsimd.dma_start(wt[:], wqkv[s * 128:(s + 1) * 128, :])
                    w_sb.append(wt)

                for lc in range(NLC):
                    lsl = bass.ts(lc, 512)
                    proj_slots = [SLOT_K, SLOT_Q0, 3, 4, 5]
                    proj_ps = {
                        s: p1ps.tile([128, 512], F32, tag=f"proj{s}", name=f"proj{s}_{lc}")
                        for s in proj_slots
                    }
                    xts = []  # xts[dq] holds dim-chunks (2dq, 2dq+1) side by side
                    for dc in range(ND):
                        dq, half = dc // 2, dc % 2
                        if half == 0:
                            xt_t = xtpool.tile(
                                [128, 1024], BF16, tag="xt", name=f"xt{dq}_{lc}"
                            )
                            nc.sync.dma_start(
                                xt_t[:],
                                xT[dq * 256:(dq + 1) * 256, lsl].rearrange(
                                    "(d p) n -> p d n", p=128
                                ),
                            )
                            xts.append(xt_t)
                        for s in proj_slots:
                            nc.tensor.matmul(
                                proj_ps[s][:],
                                w_sb[s][:, bass.ts(dc, 128)],
                                xts[dq][:, bass.ts(half, 512)],
                                start=(dc == 0),
                                stop=(dc == ND - 1),
                            )
                    # V: [seq, head_dim] layout -> lhsT = xT chunk, rhs = wv chunk
                    for jj in range(4):
                        j = lc * 4 + jj
                        v_ps = p1ps.tile([128, DH], F32, tag="vps", bufs=1, name=f"vps{j}")
                        for dc in range(ND):
                            off = (dc % 2) * 512 + jj * 128
                            nc.tensor.matmul(
                                v_ps[:],
                                xts[dc // 2][:, off:off + 128],
                                w_sb[SLOT_V][:, bass.ts(dc, 128)],
                                start=(dc == 0),
                                stop=(dc == ND - 1),
                            )
                        nc.vector.tensor_copy(v_sb[j][:], v_ps[:])

                    # RoPE: out = cos*x + sin_signed*swap(x), K and Q0 first
                    for s in proj_slots:
                        ps = proj_ps[s]
                        shuf = rwork.tile([128, 512], F32, tag="shuf", name=f"sh{s}_{lc}")
                        nc.vector.stream_shuffle(shuf[:], ps[:], swap_mask)
                        qc = rwork.tile([128, 512], F32, tag="qc", name=f"qc{s}_{lc}")
                        nc.vector.tensor_mul(qc[:], ps[:], cos_sb[:, lsl])
                        qs = rwork.tile([128, 512], F32, tag="qs", name=f"qs{s}_{lc}")
                        nc.vector.tensor_mul(qs[:], shuf[:], sin_sb[:, lsl])
                        nc.vector.tensor_add(rope_dst[s][:, lsl], qc[:], qs[:])
                if debug:
                    for s, t in enumerate([krope] + qrope):
                        nc.sync.dma_start(dbg_qk[s * 128:(s + 1) * 128, :], t[:])
                    for j in range(NJ):
                        nc.sync.dma_start(dbg_v[j * 128:(j + 1) * 128, :], v_sb[j][:])

            # ---------------- Phase 2: attention, head-pipelined ----------------
            sends, recvs = [], []
            for g, grp in enumerate(A2A_GROUPS):
                sends.append(
                    dram.tile([NCORES * len(grp) * DH, IS], BF16, name=f"send{g}")
                )
                recvs.append(
                    dram.tile([NCORES * len(grp) * DH, IS], BF16, name=f"recv{g}")
                )

            with (
                # phase-3 pools open FIRST so their SBUF is disjoint from the
                # attention pools: og/wo loads then never WAR-block on expst
                tc.tile_pool(name="og", bufs=1) as ogpool,
                tc.tile_pool(name="wos", bufs=8) as wopool,
                tc.tile_pool(name="ysb", bufs=4) as ypool,
                tc.tile_pool(name="expst", bufs=26) as epool,
                tc.tile_pool(name="otsb", bufs=2) as otpool,
                tc.tile_pool(name="nrm", bufs=6) as nrmpool,
            ):
              with tc.tile_pool(name="p2psum", bufs=1, space="PSUM") as p2ps:
                expst_of = {}

                def s_phase(h):
                    expst = []
                    for j in range(NJ):
                        et = epool.tile([128, L], BF16, tag="e", name=f"e{h}_{j}")
                        s_ps = p2ps.tile(
                            [128, 2048], F32, tag="s", bufs=1, name=f"s{h}_{j}"
                        )
                        for i in range(4):
                            nc.tensor.matmul(
                                s_ps[:, bass.ts(i, 512)],
                                krope[:, bass.ts(j, 128)],
                                qrope[h][:, bass.ts(i, 512)],
                                start=True,
                                stop=True,
                            )
                        nc.scalar.activation(
                            et[:],
                            s_ps[:],
                            mybir.ActivationFunctionType.Exp,
                            scale=SCALE,
                        )
                        expst.append(et)
                    expst_of[h] = expst

                def av_phase(h):
                    expst = expst_of.pop(h)
                    rb_sbs = []
                    for i in range(NI):
                        isl = bass.ts(i, 512)
                        sums_ps = p2ps.tile(
                            [128, 512], F32, tag="small", bufs=2, name=f"sm{h}_{i}"
                        )
                        for j in range(NJ):
                            nc.tensor.matmul(
                                sums_ps[:],
                                ones_bc[:],
                                expst[j][:, isl],
                                start=(j == 0),
                                stop=(j == NJ - 1),
                            )
                        rb = nrmpool.tile([128, 512], F32, tag="rb", name=f"rb{h}_{i}")
                        nc.vector.reciprocal(rb[:], sums_ps[:])
                        rb_sbs.append(rb)
                    ot_sb = otpool.tile([128, L], BF16, tag="ot", name=f"ot{h}")
                    for i in range(NI):
                        isl = bass.ts(i, 512)
                        ot_ps = p2ps.tile(
                            [128, 512], F32, tag="ot", bufs=2, name=f"otp{h}_{i}"
                        )
                        for j in range(NJ):
                            nc.tensor.matmul(
                                ot_ps[:],
                                v_sb[j][:],
                                expst[j][:, isl],
                                start=(j == 0),
                                stop=(j == NJ - 1),
                            )
                        nc.vector.tensor_mul(ot_sb[:, isl], ot_ps[:], rb_sbs[i][:])
                    # scatter into A2A send buffer
                    g = next(i for i, grp in enumerate(A2A_GROUPS) if h in grp)
                    grp = A2A_GROUPS[g]
                    hh, nh = grp.index(h), len(grp)
                    for c in range(NCORES):
                        nc.gpsimd.dma_start(
                            sends[g][(c * nh + hh) * 128:(c * nh + hh + 1) * 128, :],
                            ot_sb[:, c * IS:(c + 1) * IS],
                        )
                    if h == grp[-1]:
                        nc.gpsimd.collective_compute(
                            "AllToAll",
                            mybir.AluOpType.bypass,
                            replica_groups=[list(range(NCORES))],
                            ins=[sends[g][:].opt()],
                            outs=[recvs[g][:].opt()],
                        )

                for h in range(HQ):
                    s_phase(h)
                    if h > 0:
                        av_phase(h - 1)
                av_phase(HQ - 1)

              # ---------------- Phase 3: output projection ----------------
              with tc.tile_pool(name="p3psum", bufs=1, space="PSUM") as p3ps:
                # one batched load per A2A group: og_all[:, t*256:(t+1)*256]
                # holds hd-chunk t in [128, 256] layout
                og_all = ogpool.tile([128, ND * IS], BF16)
                tbase = 0
                for g, grp in enumerate(A2A_GROUPS):
                    ngt = NCORES * len(grp)
                    # late groups go via the vector queue so they can't
                    # head-of-line block the wo weight stream
                    eng = nc.scalar if g == 0 else nc.sync
                    eng.dma_start(
                        og_all[:, tbase * IS:(tbase + ngt) * IS],
                        recvs[g][:].rearrange("(t p) i -> p t i", p=128),
                    )
                    tbase += ngt
                if debug:
                    nc.sync.dma_start(
                        dbg_og[:].rearrange("(t p) i -> p t i", p=128), og_all[:]
                    )

                for dob in range(2):
                    y_ps = [
                        [
                            p3ps.tile([128, 512], F32, tag=f"y{d2}_{ii}", name=f"y{dob}_{d2}_{ii}")
                            for ii in range(2)
                        ]
                        for d2 in range(4)
                    ]
                    for tq in range(8):
                        wo_ts = []
                        for d2 in range(4):
                            do = dob * 4 + d2
                            wo_t = wopool.tile([128, 4 * 512], BF16, tag="wo", name=f"wo{do}_{tq}")
                            eng = nc.scalar if d2 % 2 == 0 else nc.gpsimd
                            eng.dma_start(
                                wo_t[:],
                                wo[do * 8 + tq, :].rearrange("(p n) -> p n", p=128),
                            )
                            wo_ts.append(wo_t)
                        for tt in range(4):
                            t = tq * 4 + tt
                            for d2 in range(4):
                                for ii in range(2):
                                    nc.tensor.matmul(
                                        y_ps[d2][ii][:],
                                        og_all[:, t * IS + ii * 128:t * IS + (ii + 1) * 128],
                                        wo_ts[d2][:, bass.ts(tt, 512)],
                                        start=(t == 0),
                                        stop=(t == ND - 1),
                                    )
                    for d2 in range(4):
                        do = dob * 4 + d2
                        for ii in range(2):
                            y_sb = ypool.tile([128, 512], F32, tag="y", name=f"ys{do}_{ii}")
                            nc.vector.tensor_copy(y_sb[:], y_ps[d2][ii][:])
                            nc.scalar.dma_start(
                                out[ii * 128:(ii + 1) * 128, bass.ts(do, 512)], y_sb[:]
                            )

    nc.compile()
    return nc


def _rope_tables(seq_len):
    inv_freq = 1.0 / (ROPE_THETA ** (np.arange(0, DH, 2, dtype=np.float32) / DH))
    t = np.arange(seq_len, dtype=np.float32)
    freqs = t[:, None] * inv_freq[None, :]
    emb = np.concatenate([freqs, freqs], axis=-1)  # [L, DH]
    cos_e = np.cos(emb)
    sin_e = np.sin(emb)
    sign = np.where(np.arange(DH) % 2 == 0, np.float32(-1.0), np.float32(1.0))
    return cos_e.T.copy(), (sin_e * sign[None, :]).T.copy()  # [DH, L] each


def _prep_in_maps(x, wq, wk, wv, wo, seq_len):
    bf = ml_dtypes.bfloat16
    xT = np.ascontiguousarray(np.asarray(x, np.float32).reshape(L, D).T).astype(bf)
    cosT, sinT = _rope_tables(int(seq_len))

    # hd-contraction order matching A2A arrival: per group, core-major.
    perm = [
        c * HQ + h for grp in A2A_GROUPS for c in range(NCORES) for h in grp
    ]
    wo_b = (
        np.asarray(wo, np.float32)
        .reshape(ND, 128, 8, 512)
        .transpose(2, 0, 1, 3)[:, perm]          # [do, t, p, n]
        .reshape(8, 8, 4, 128, 512)              # [do, tq, tt, p, n]
        .transpose(0, 1, 3, 2, 4)                # [do, tq, p, tt, n]
        .reshape(8 * 8, 128 * 4 * 512)
        .astype(bf)
    )

    def head_tile(w2d):  # [D, 128] -> [128, ND*128] p-major tiling
        return (
            np.asarray(w2d, np.float32)
            .reshape(ND, 128, 128)
            .transpose(1, 0, 2)
            .reshape(128, ND * 128)
            .astype(bf)
        )

    in_maps = []
    for r in range(NCORES):
        q_tiles = [
            head_tile(wq[:, (HQ * r + h) * DH:(HQ * r + h + 1) * DH]) for h in range(HQ)
        ]
        # slot order: k, q0, v, q1, q2, q3
        slots = [
            head_tile(wk[:, r * DH:(r + 1) * DH]),
            q_tiles[0],
            head_tile(wv[:, r * DH:(r + 1) * DH]),
            q_tiles[1],
            q_tiles[2],
            q_tiles[3],
        ]
        in_maps.append(
            {
                "xT": xT,
                "wqkv": np.concatenate(slots, axis=0),
                "wo": wo_b,
                "cosT": cosT,
                "sinT": sinT,
            }
        )
    return in_maps


def kernel(x, wq, wk, wv, wo, seq_len):
    import time

    if "nc" not in _cached:
        _cached["nc"] = build_kernel()
    nc = _cached["nc"]
    in_maps = _prep_in_maps(x, wq, wk, wv, wo, seq_len)
    last_exc = None
    for attempt in range(3):
        try:
            res = bass_utils.run_bass_kernel_spmd(
                nc, in_maps, core_ids=list(range(NCORES))
            )
            break
        except Exception as e:  # transient NRT_EXEC_UNIT_UNRECOVERABLE flakes
            last_exc = e
            time.sleep(10)
    else:
        raise last_exc
    _cached["last_results"] = res
    y = np.concatenate([res.results[r]["out"] for r in range(NCORES)], axis=0)
    return y.reshape(1, L, D).astype(np.float32)


# revision 66
# speedup vs baseline: 1.1142x; 1.1142x over previous
"""Distributed GQA attention block (dense transformer) on 8 TRN2 NeuronCores.

Strategy: tensor-parallel over heads. Each core owns 4 query heads + 1 KV head
(GQA group). x^T is replicated; Q/K/V projections, RoPE, scores, softmax and
the attention output all stay in "transposed" layout (feature dim on SBUF
partitions, sequence on the free dim) so no on-device transposes are needed.
The per-core attention outputs are exchanged with AllToAll collectives (each
core keeps a 256-row slice of the sequence), then each core computes its slice
of the output projection against the full (replicated, pre-tiled) wo. The host
concatenates the 8 row slices.

All matmuls run in bf16 with fp32 PSUM accumulation; softmax exp runs in fp32
on the scalar engine (no max-subtraction needed: |scores*scale| <~ 12).

Schedule notes:
- heads are software-pipelined: the sums/AV matmuls of head h-1 are emitted
  after the score matmuls of head h, so the scalar engine's exp of head h
  overlaps PE work of head h-1.
- the AllToAll is split: heads 0-2 exchange while head 3 computes; head 3's
  exchange is covered by the first 3/4 of the output-projection matmuls
  (the wo contraction order is host-permuted to put head-3 blocks last).
"""

import numpy as np
import ml_dtypes

import concourse.bass as bass
import concourse.mybir as mybir
import concourse.tile as tile
from concourse import bacc
from concourse import bass_utils

F32 = mybir.dt.float32
BF16 = mybir.dt.bfloat16

# Problem shape (hardcoded per harness contract).
L = 2048          # sequence length
D = 4096          # model dim
DH = 128          # head dim
NHEADS = 32
NKV = 8
NCORES = 8
HQ = NHEADS // NCORES      # 4 query heads per core
ROPE_THETA = 10000.0
SCALE = DH ** -0.5

ND = D // 128              # 32 contraction chunks over model dim
NLC = L // 512             # 4 free-dim chunks of 512 over sequence
NJ = L // 128              # 16 key chunks of 128
NI = L // 512              # 4 query chunks of 512
IS = L // NCORES           # 256: per-core output row slice

# AllToAll groups: heads {0,1} fly during head-2 compute, {2} during head-3,
# {3} is covered by the first 3/4 of the output projection.
A2A_GROUPS = [(0, 1), (2,), (3,)]

_cached = {}


def build_kernel(debug=False):
    nc = bacc.Bacc(num_devices=NCORES)

    xT = nc.dram_tensor("xT", [D, L], BF16, kind="ExternalInput")
    # 6 head-slots in compute order (k, q0, v, q1, q2, q3), each pre-tiled to
    # [128 partitions, 32*128]: [:, dc*128:(dc+1)*128] is dim-chunk dc.
    wqkv = nc.dram_tensor("wqkv", [6 * 128, ND * 128], BF16, kind="ExternalInput")
    # wo pre-tiled: row (do*8+tq) is a fully contiguous [128, 4*512] SBUF
    # image covering hd-chunks perm[4tq..4tq+3] for out-column chunk do
    # (perm = A2A arrival order).
    wo = nc.dram_tensor("wo", [8 * 8, 128 * 4 * 512], BF16, kind="ExternalInput")
    cosT = nc.dram_tensor("cosT", [128, L], F32, kind="ExternalInput")
    sinT = nc.dram_tensor("sinT", [128, L], F32, kind="ExternalInput")  # sign-folded
    out = nc.dram_tensor("out", [IS, D], F32, kind="ExternalOutput")
    if debug:
        dbg_qk = nc.dram_tensor("dbg_qk", [5 * 128, L], BF16, kind="ExternalOutput")
        dbg_v = nc.dram_tensor("dbg_v", [NJ * 128, DH], BF16, kind="ExternalOutput")
        dbg_og = nc.dram_tensor("dbg_og", [NCORES * HQ * DH, IS], BF16, kind="ExternalOutput")

    swap_mask = []
    for i in range(16):
        swap_mask += [2 * i + 1, 2 * i]

    # slot order in wqkv / processing: k, q0, v, q1, q2, q3
    SLOT_K, SLOT_Q0, SLOT_V = 0, 1, 2
    slot_of_head = [1, 3, 4, 5]  # q0..q3

    with tile.TileContext(nc) as tc:
        with (
            tc.tile_pool(name="const", bufs=1) as cpool,
            tc.tile_pool(name="persist", bufs=1) as ppool,
            tc.tile_pool(name="dram", bufs=1, space="DRAM") as dram,
        ):
            ones_bc = cpool.tile([128, 128], BF16)
            nc.vector.memset(ones_bc[:], 1.0)

            # Roped K^T + Q^T (4 heads), bf16, [head_dim=128, L]
            qk_rope = [ppool.tile([128, L], BF16, name=f"qkrope{s}") for s in range(5)]
            krope = qk_rope[0]
            qrope = [qk_rope[1], qk_rope[2], qk_rope[3], qk_rope[4]]
            rope_dst = {SLOT_K: krope, 3: qrope[1], 4: qrope[2], 5: qrope[3],
                        SLOT_Q0: qrope[0]}
            # V in [seq, head_dim] layout: 16 chunks of [128, 128]
            v_sb = [ppool.tile([128, DH], BF16, name=f"vsb{j}") for j in range(NJ)]

            # ---------------- Phase 1: projections + rope ----------------
            with (
                tc.tile_pool(name="tbl", bufs=1) as tblpool,
                tc.tile_pool(name="wq", bufs=1) as wpool,
                tc.tile_pool(name="xt", bufs=18) as xtpool,
                tc.tile_pool(name="p1psum", bufs=1, space="PSUM") as p1ps,
                tc.tile_pool(name="ropework", bufs=3) as rwork,
            ):
                # weight/table loads go through gpsimd's queue so the xt loads
                # on the sync queue aren't stuck behind them at startup
                cos_sb = tblpool.tile([128, L], F32)
                sin_sb = tblpool.tile([128, L], F32)
                nc.gpsimd.dma_start(cos_sb[:], cosT[:])
                nc.gpsimd.dma_start(sin_sb[:], sinT[:])
                w_sb = []
                for s in range(6):
                    wt = wpool.tile([128, ND * 128], BF16, name=f"w{s}")
                    if s < 2:
                        # chunked so the first matmuls unblock early
                        for q in range(4):
                            nc.gpsimd.dma_start(
                                wt[:, bass.ts(q, ND * 32)],
                                wqkv[s * 128:(s + 1) * 128, bass.ts(q, ND * 32)],
                            )
                    else:
                        nc.gpsimd.dma_start(wt[:], wqkv[s * 128:(s + 1) * 128, :])
                    w_sb.append(wt)

                for lc in range(NLC):
                    lsl = bass.ts(lc, 512)
                    proj_slots = [SLOT_K, SLOT_Q0, 3, 4, 5]
                    proj_ps = {
                        s: p1ps.tile([128, 512], F32, tag=f"proj{s}", name=f"proj{s}_{lc}")
                        for s in proj_slots
                    }
                    xts = []  # xts[dq] holds dim-chunks (2dq, 2dq+1) side by side
                    for dc in range(ND):
                        dq, half = dc // 2, dc % 2
                        if half == 0:
                            xt_t = xtpool.tile(
                                [128, 1024], BF16, tag="xt", name=f"xt{dq}_{lc}"
                            )
                            nc.sync.dma_start(
                                xt_t[:],
                                xT[dq * 256:(dq + 1) * 256, lsl].rearrange(
                                    "(d p) n -> p d n", p=128
                                ),
                            )
                            xts.append(xt_t)
                        for s in proj_slots:
                            nc.tensor.matmul(
                                proj_ps[s][:],
                                w_sb[s][:, bass.ts(dc, 128)],
                                xts[dq][:, bass.ts(half, 512)],
                                start=(dc == 0),
                                stop=(dc == ND - 1),
                            )
                    # V: [seq, head_dim] layout -> lhsT = xT chunk, rhs = wv chunk
                    for jj in range(4):
                        j = lc * 4 + jj
                        v_ps = p1ps.tile([128, DH], F32, tag="vps", bufs=1, name=f"vps{j}")
                        for dc in range(ND):
                            off = (dc % 2) * 512 + jj * 128
                            nc.tensor.matmul(
                                v_ps[:],
                                xts[dc // 2][:, off:off + 128],
                                w_sb[SLOT_V][:, bass.ts(dc, 128)],
                                start=(dc == 0),
                                stop=(dc == ND - 1),
                            )
                        nc.vector.tensor_copy(v_sb[j][:], v_ps[:])

                    # RoPE: out = cos*x + sin_signed*swap(x), K and Q0 first
                    for s in proj_slots:
                        ps = proj_ps[s]
                        shuf = rwork.tile([128, 512], F32, tag="shuf", name=f"sh{s}_{lc}")
                        nc.vector.stream_shuffle(shuf[:], ps[:], swap_mask)
                        qc = rwork.tile([128, 512], F32, tag="qc", name=f"qc{s}_{lc}")
                        nc.vector.tensor_mul(qc[:], ps[:], cos_sb[:, lsl])
                        qs = rwork.tile([128, 512], F32, tag="qs", name=f"qs{s}_{lc}")
                        nc.vector.tensor_mul(qs[:], shuf[:], sin_sb[:, lsl])
                        nc.vector.tensor_add(rope_dst[s][:, lsl], qc[:], qs[:])
                if debug:
                    for s, t in enumerate([krope] + qrope):
                        nc.sync.dma_start(dbg_qk[s * 128:(s + 1) * 128, :], t[:])
                    for j in range(NJ):
                        nc.sync.dma_start(dbg_v[j * 128:(j + 1) * 128, :], v_sb[j][:])

            # ---------------- Phase 2: attention, head-pipelined ----------------
            sends, recvs = [], []
            for g, grp in enumerate(A2A_GROUPS):
                sends.append(
                    dram.tile([NCORES * len(grp) * DH, IS], BF16, name=f"send{g}")
                )
                recvs.append(
                    dram.tile([NCORES * len(grp) * DH, IS], BF16, name=f"recv{g}")
                )

            with (
                # phase-3 pools open FIRST so their SBUF is disjoint from the
                # attention pools: og/wo loads then never WAR-block on expst
                tc.tile_pool(name="og", bufs=1) as ogpool,
                tc.tile_pool(name="wos", bufs=8) as wopool,
                tc.tile_pool(name="ysb", bufs=4) as ypool,
                tc.tile_pool(name="expst", bufs=26) as epool,
                tc.tile_pool(name="otsb", bufs=2) as otpool,
                tc.tile_pool(name="nrm", bufs=6) as nrmpool,
            ):
              with tc.tile_pool(name="p2psum", bufs=1, space="PSUM") as p2ps:
                expst_of = {}

                def s_phase(h):
                    expst = []
                    for j in range(NJ):
                        et = epool.tile([128, L], BF16, tag="e", name=f"e{h}_{j}")
                        for ih in range(2):
                            s_ps = p2ps.tile(
                                [128, 1024], F32, tag="s", bufs=2, name=f"s{h}_{j}_{ih}"
                            )
                            for i2 in range(2):
                                i = ih * 2 + i2
                                nc.tensor.matmul(
                                    s_ps[:, bass.ts(i2, 512)],
                                    krope[:, bass.ts(j, 128)],
                                    qrope[h][:, bass.ts(i, 512)],
                                    start=True,
                                    stop=True,
                                )
                            nc.scalar.activation(
                                et[:, bass.ts(ih, 1024)],
                                s_ps[:],
                                mybir.ActivationFunctionType.Exp,
                                scale=SCALE,
                            )
                        expst.append(et)
                    expst_of[h] = expst

                def av_phase(h):
                    expst = expst_of.pop(h)
                    rb_sbs = []
                    for i in range(NI):
                        isl = bass.ts(i, 512)
                        sums_ps = p2ps.tile(
                            [128, 512], F32, tag="small", bufs=2, name=f"sm{h}_{i}"
                        )
                        for j in range(NJ):
                            nc.tensor.matmul(
                                sums_ps[:],
                                ones_bc[:],
                                expst[j][:, isl],
                                start=(j == 0),
                                stop=(j == NJ - 1),
                            )
                        rb = nrmpool.tile([128, 512], F32, tag="rb", name=f"rb{h}_{i}")
                        nc.vector.reciprocal(rb[:], sums_ps[:])
                        rb_sbs.append(rb)
                    ot_sb = otpool.tile([128, L], BF16, tag="ot", name=f"ot{h}")
                    for i in range(NI):
                        isl = bass.ts(i, 512)
                        ot_ps = p2ps.tile(
                            [128, 512], F32, tag="ot", bufs=2, name=f"otp{h}_{i}"
                        )
                        for j in range(NJ):
                            nc.tensor.matmul(
                                ot_ps[:],
                                v_sb[j][:],
                                expst[j][:, isl],
                                start=(j == 0),
                                stop=(j == NJ - 1),
                            )
                        nc.vector.tensor_mul(ot_sb[:, isl], ot_ps[:], rb_sbs[i][:])
                    # scatter into A2A send buffer
                    g = next(i for i, grp in enumerate(A2A_GROUPS) if h in grp)
                    grp = A2A_GROUPS[g]
                    hh, nh = grp.index(h), len(grp)
                    for c in range(NCORES):
                        nc.gpsimd.dma_start(
                            sends[g][(c * nh + hh) * 128:(c * nh + hh + 1) * 128, :],
                            ot_sb[:, c * IS:(c + 1) * IS],
                        )
                    if h == grp[-1]:
                        nc.gpsimd.collective_compute(
                            "AllToAll",
                            mybir.AluOpType.bypass,
                            replica_groups=[list(range(NCORES))],
                            ins=[sends[g][:].opt()],
                            outs=[recvs[g][:].opt()],
                        )

                for h in range(HQ):
                    s_phase(h)
                    if h > 0:
                        av_phase(h - 1)
                av_phase(HQ - 1)

              # ---------------- Phase 3: output projection ----------------
              with tc.tile_pool(name="p3psum", bufs=1, space="PSUM") as p3ps:
                # one batched load per A2A group: og_all[:, t*256:(t+1)*256]
                # holds hd-chunk t in [128, 256] layout
                og_all = ogpool.tile([128, ND * IS], BF16)
                tbase = 0
                for g, grp in enumerate(A2A_GROUPS):
                    ngt = NCORES * len(grp)
                    # late groups go via the vector queue so they can't
                    # head-of-line block the wo weight stream
                    eng = nc.scalar if g == 0 else nc.sync
                    eng.dma_start(
                        og_all[:, tbase * IS:(tbase + ngt) * IS],
                        recvs[g][:].rearrange("(t p) i -> p t i", p=128),
                    )
                    tbase += ngt
                if debug:
                    nc.sync.dma_start(
                        dbg_og[:].rearrange("(t p) i -> p t i", p=128), og_all[:]
                    )

                for dob in range(2):
                    y_ps = [
                        [
                            p3ps.tile([128, 512], F32, tag=f"y{d2}_{ii}", name=f"y{dob}_{d2}_{ii}")
                            for ii in range(2)
                        ]
                        for d2 in range(4)
                    ]
                    for tq in range(8):
                        wo_ts = []
                        for d2 in range(4):
                            do = dob * 4 + d2
                            wo_t = wopool.tile([128, 4 * 512], BF16, tag="wo", name=f"wo{do}_{tq}")
                            eng = nc.scalar if d2 % 2 == 0 else nc.gpsimd
                            eng.dma_start(
                                wo_t[:],
                                wo[do * 8 + tq, :].rearrange("(p n) -> p n", p=128),
                            )
                            wo_ts.append(wo_t)
                        for tt in range(4):
                            t = tq * 4 + tt
                            for d2 in range(4):
                                for ii in range(2):
                                    nc.tensor.matmul(
                                        y_ps[d2][ii][:],
                                        og_all[:, t * IS + ii * 128:t * IS + (ii + 1) * 128],
                                        wo_ts[d2][:, bass.ts(tt, 512)],
                                        start=(t == 0),
                                        stop=(t == ND - 1),
                                    )
                    for d2 in range(4):
                        do = dob * 4 + d2
                        for ii in range(2):
                            y_sb = ypool.tile([128, 512], F32, tag="y", name=f"ys{do}_{ii}")
                            nc.vector.tensor_copy(y_sb[:], y_ps[d2][ii][:])
                            nc.scalar.dma_start(
                                out[ii * 128:(ii + 1) * 128, bass.ts(do, 512)], y_sb[:]
                            )

    nc.compile()
    return nc


def _rope_tables(seq_len):
    inv_freq = 1.0 / (ROPE_THETA ** (np.arange(0, DH, 2, dtype=np.float32) / DH))
    t = np.arange(seq_len, dtype=np.float32)
    freqs = t[:, None] * inv_freq[None, :]
    emb = np.concatenate([freqs, freqs], axis=-1)  # [L, DH]
    cos_e = np.cos(emb)
    sin_e = np.sin(emb)
    sign = np.where(np.arange(DH) % 2 == 0, np.float32(-1.0), np.float32(1.0))
    return cos_e.T.copy(), (sin_e * sign[None, :]).T.copy()  # [DH, L] each


def _prep_in_maps(x, wq, wk, wv, wo, seq_len):
    bf = ml_dtypes.bfloat16
    xT = np.ascontiguousarray(np.asarray(x, np.float32).reshape(L, D).T).astype(bf)
    cosT, sinT = _rope_tables(int(seq_len))

    # hd-contraction order matching A2A arrival: per group, core-major.
    perm = [
        c * HQ + h for grp in A2A_GROUPS for c in range(NCORES) for h in grp
    ]
    wo_b = (
        np.asarray(wo, np.float32)
        .reshape(ND, 128, 8, 512)
        .transpose(2, 0, 1, 3)[:, perm]          # [do, t, p, n]
        .reshape(8, 8, 4, 128, 512)              # [do, tq, tt, p, n]
        .transpose(0, 1, 3, 2, 4)                # [do, tq, p, tt, n]
        .reshape(8 * 8, 128 * 4 * 512)
        .astype(bf)
    )

    def head_tile(w2d):  # [D, 128] -> [128, ND*128] p-major tiling
        return (
            np.asarray(w2d, np.float32)
            .reshape(ND, 128, 128)
            .transpose(1, 0, 2)
            .reshape(128, ND * 128)
            .astype(bf)
        )

    in_maps = []
    for r in range(NCORES):
        q_tiles = [
            head_tile(wq[:, (HQ * r + h) * DH:(HQ * r + h + 1) * DH]) for h in range(HQ)
        ]
        # slot order: k, q0, v, q1, q2, q3
        slots = [
            head_tile(wk[:, r * DH:(r + 1) * DH]),
            q_tiles[0],
            head_tile(wv[:, r * DH:(r + 1) * DH]),
            q_tiles[1],
            q_tiles[2],
            q_tiles[3],
        ]
        in_maps.append(
            {
                "xT": xT,
                "wqkv": np.concatenate(slots, axis=0),
                "wo": wo_b,
                "cosT": cosT,
                "sinT": sinT,
            }
        )
    return in_maps


def kernel(x, wq, wk, wv, wo, seq_len):
    import time

    if "nc" not in _cached:
        _cached["nc"] = build_kernel()
    nc = _cached["nc"]
    in_maps = _prep_in_maps(x, wq, wk, wv, wo, seq_len)
    last_exc = None
    for attempt in range(3):
        try:
            res = bass_utils.run_bass_kernel_spmd(
                nc, in_maps, core_ids=list(range(NCORES))
            )
            break
        except Exception as e:  # transient NRT_EXEC_UNIT_UNRECOVERABLE flakes
            last_exc = e
            time.sleep(10)
    else:
        raise last_exc
    _cached["last_results"] = res
    y = np.concatenate([res.results[r]["out"] for r in range(NCORES)], axis=0)
    return y.reshape(1, L, D).astype(np.float32)
